# revision 18
# baseline (speedup 1.0000x reference)
"""DecisionTransformer forward on 8 TRN2 NeuronCores.

Strategy: data-parallel over batch (32 -> 4 per core), weights replicated.
Host computes the tiny embedding front-end (token/state/action/reward embeds,
interleave, ln0) and the tiny prediction heads; the 8 transformer blocks
(>99% of FLOPs) run on-device in one Bass/Tile kernel per core.

Device layout: activations are kept feature-major ("transposed"): xT[H, tok]
with H=768 on 6 partition-tiles of 128 and tok = 4 batches x 384 tokens.
All matmuls consume/produce this layout directly. LayerNorm (reduction over
the partition dim) uses ones-matmul stats + rank-1 PE broadcasts.
Attention uses scores in [j, i] orientation with exp-without-max (scores are
O(1) here), multiplicative causal mask on the diagonal block only, and a
ones-column in V to get softmax denominators for free.

The math shortcut vs the reference: the expert blocks' prompt tokens are
appended AFTER the sequence and causally masked, and their outputs are
discarded -> they cannot affect the kept outputs. So all 8 blocks are plain
causal blocks over 384 tokens.
"""

import numpy as np
import ml_dtypes

import concourse.bass as bass
import concourse.tile as tile
from concourse import bacc, mybir
from concourse import bass_utils

# ---- model constants (hardcoded per spec) ----
B, T = 32, 128
H = 768
NH = 12
HD = 64          # head dim
FF = 4 * H       # 3072
L = 8            # 6 blk + 2 expert blocks
S = 3 * T        # 384 tokens per sequence
NCORES = 8
BPC = B // NCORES          # 4 batches per core
TOK = BPC * S              # 1536 tokens per core
P = 128
HT = H // P                # 6 partition tiles for H
FFT = FF // P              # 24 partition tiles for FF
SB = S // P                # 3 j-blocks per sequence
EPS = 1e-5
ATT_SCALE = 1.0 / np.sqrt(HD)
NCH = 3                    # token chunks
CW = TOK // NCH            # 512

BF16 = mybir.dt.bfloat16
F32 = mybir.dt.float32
np_bf16 = ml_dtypes.bfloat16

_CACHE = {}


def _build():
    """Build + compile the per-core Bass program (same NEFF on all 8 cores)."""
    nc = bacc.Bacc("TRN2", target_bir_lowering=False, debug=False,
                   enable_asserts=True, num_devices=NCORES)

    # ---- DRAM parameters ----
    x0T = nc.dram_tensor("x0T", [H, TOK], BF16, kind="ExternalInput").ap()
    wq = nc.dram_tensor("wq", [L, H, H], BF16, kind="ExternalInput").ap()
    wk = nc.dram_tensor("wk", [L, H, H], BF16, kind="ExternalInput").ap()
    wv = nc.dram_tensor("wv", [L, H, H], BF16, kind="ExternalInput").ap()
    wo = nc.dram_tensor("wo", [L, H, H], BF16, kind="ExternalInput").ap()
    w1 = nc.dram_tensor("w1", [L, H, FF], BF16, kind="ExternalInput").ap()
    w2 = nc.dram_tensor("w2", [L, FF, H], BF16, kind="ExternalInput").ap()
    bq = nc.dram_tensor("bq", [L, H], F32, kind="ExternalInput").ap()
    bk = nc.dram_tensor("bk", [L, H], F32, kind="ExternalInput").ap()
    bo2 = nc.dram_tensor("bo2", [L, H], F32, kind="ExternalInput").ap()  # bv@Wo+bo
    b1 = nc.dram_tensor("b1", [L, FF], F32, kind="ExternalInput").ap()
    b2 = nc.dram_tensor("b2", [L, H], F32, kind="ExternalInput").ap()
    # ln params as bf16 rows (feed PE rank-1 broadcast matmuls)
    ln_s = nc.dram_tensor("ln_s", [L, 2, H], BF16, kind="ExternalInput").ap()
    ln_b = nc.dram_tensor("ln_b", [L, 2, H], BF16, kind="ExternalInput").ap()
    # multiplicative causal mask, [j-block, pj, i] (diagonal block only is used)
    maskT = nc.dram_tensor("maskT", [SB, P, S], BF16, kind="ExternalInput").ap()
    hout = nc.dram_tensor("houtT", [H, TOK], F32, kind="ExternalOutput").ap()

    x0T_t = x0T.rearrange("(t p) n -> p t n", p=P)
    hout_t = hout.rearrange("(t p) n -> p t n", p=P)

    with tile.TileContext(nc) as tc:
        with tc.tile_pool(name="glob", bufs=1) as glob, \
             tc.tile_pool(name="xpool", bufs=2) as xpool, \
             tc.tile_pool(name="whh", bufs=3) as whhp, \
             tc.tile_pool(name="wbig", bufs=2) as wbigp, \
             tc.tile_pool(name="rows", bufs=1) as rows:

            # constants
            ones_col = glob.tile([P, 1], BF16, tag="onec")
            nc.vector.memset(ones_col[:], 1.0)
            ones_row = glob.tile([1, TOK], BF16, tag="oner")
            nc.vector.memset(ones_row[:], 1.0)
            mask_sb = glob.tile([P, SB, S], BF16, tag="mask")
            nc.sync.dma_start(mask_sb[:], maskT.rearrange("j p i -> p j i"))
            eps_c = glob.tile([1, 1], F32, tag="eps")
            nc.vector.memset(eps_c[:], EPS)

            # initial activations
            x = xpool.tile([P, HT, TOK], BF16, tag="x")
            nc.sync.dma_start(x[:], x0T_t)

            def load_whh(ap_l, nm):
                t = whhp.tile([P, HT, H], BF16, tag="whh", name=nm)
                nc.sync.dma_start(t[:], ap_l.rearrange("(ko ki) m -> ki ko m", ki=P))
                return t

            def load_bias(ap_l, n, nm):  # [n*P] f32 -> [P, n]
                t = rows.tile([P, n], F32, tag=f"b_{nm}", bufs=2, name=nm)
                nc.sync.dma_start(t[:], ap_l.rearrange("(t p) -> p t", p=P))
                return t

            def layernorm(l, which, x_in, ctx_name, final_dma=None):
                """x_in: [P, HT, TOK] bf16 -> new x tile (normalized, *s+b)."""
                s_row = rows.tile([1, H], BF16, tag="s_row", bufs=2, name="s_row")
                nc.sync.dma_start(s_row[:], ln_s[l, which][None, :])
                b_row = rows.tile([1, H], BF16, tag="b_row", bufs=2, name="b_row")
                nc.sync.dma_start(b_row[:], ln_b[l, which][None, :])
                negs_row = rows.tile([1, H], BF16, tag="negs_row", bufs=2,
                                     name="negs_row")
                nc.scalar.mul(negs_row[:], s_row[:], -1.0)
                r_bf = rows.tile([1, TOK], BF16, tag="r_bf", bufs=1, name="r_bf")
                mr_bf = rows.tile([1, TOK], BF16, tag="mr_bf", bufs=1, name="mr_bf")

                with tc.tile_pool(name=f"ln{ctx_name}", bufs=1) as lnp, \
                     tc.tile_pool(name=f"lnps{ctx_name}", bufs=2,
                                  space="PSUM") as lnps:
                    sq = lnp.tile([P, HT, TOK], BF16, tag="sq")
                    nc.scalar.square(sq[:], x_in[:])

                    for cn in range(NCH):
                        sl = bass.ts(cn, CW)
                        ps0 = lnps.tile([1, CW], F32, tag="pst0", name="ps0")
                        ps1 = lnps.tile([1, CW], F32, tag="pst1", name="ps1")
                        for k in range(HT):
                            nc.tensor.matmul(ps0[:], ones_col[:], x_in[:, k, sl],
                                             start=(k == 0), stop=(k == HT - 1))
                        for k in range(HT):
                            nc.tensor.matmul(ps1[:], ones_col[:], sq[:, k, sl],
                                             start=(k == 0), stop=(k == HT - 1))
                        mu_c = rows.tile([1, CW], F32, tag="mu_c", bufs=1,
                                         name="mu_c")
                        var_c = rows.tile([1, CW], F32, tag="var_c", bufs=1,
                                          name="var_c")
                        rr_c = rows.tile([1, CW], F32, tag="rr_c", bufs=1,
                                         name="rr_c")
                        nc.scalar.mul(mu_c[:], ps0[:], 1.0 / H)
                        nc.vector.tensor_mul(var_c[:], mu_c[:], mu_c[:])
                        nc.scalar.activation(var_c[:], var_c[:],
                                             mybir.ActivationFunctionType.Copy,
                                             scale=-1.0)
                        nc.vector.scalar_tensor_tensor(
                            out=var_c[:], in0=ps1[:], scalar=1.0 / H,
                            in1=var_c[:], op0=mybir.AluOpType.mult,
                            op1=mybir.AluOpType.add)      # E[x^2] - mu^2
                        nc.scalar.activation(var_c[:], var_c[:],
                                             mybir.ActivationFunctionType.Sqrt,
                                             bias=eps_c[:])    # std
                        nc.vector.reciprocal(rr_c[:], var_c[:])
                        nc.scalar.copy(r_bf[:, sl], rr_c[:])
                        nc.vector.tensor_mul(mr_bf[:, sl], mu_c[:], rr_c[:])

                    if final_dma is None:
                        x_out = xpool.tile([P, HT, TOK], BF16, tag="x",
                                           name="x_ln")
                    else:
                        x_out = lnp.tile([P, HT, TOK], F32, tag="xf32",
                                         name="x_f32")
                    for pt in range(HT):
                        psl = bass.ts(pt, P)
                        for cn in range(NCH):
                            sl = bass.ts(cn, CW)
                            a_ps = lnps.tile([P, CW], F32, tag="pA", name="a_ps")
                            nc.tensor.matmul(a_ps[:], s_row[:, psl], r_bf[:, sl],
                                             start=True, stop=True)
                            c_ps = lnps.tile([P, CW], F32, tag="pC", name="c_ps")
                            nc.tensor.matmul(c_ps[:], b_row[:, psl],
                                             ones_row[:, sl],
                                             start=True, stop=False)
                            nc.tensor.matmul(c_ps[:], negs_row[:, psl],
                                             mr_bf[:, sl],
                                             start=False, stop=True)
                            nc.vector.tensor_mul(x_out[:, pt, sl],
                                                 x_in[:, pt, sl], a_ps[:])
                            nc.vector.tensor_add(x_out[:, pt, sl],
                                                 x_out[:, pt, sl], c_ps[:])
                    if final_dma is not None:
                        nc.sync.dma_start(final_dma, x_out[:])
                        return None
                return x_out

            for l in range(L):
                # ---- weights for this block ----
                wq_sb = load_whh(wq[l], "wq_sb")
                wk_sb = load_whh(wk[l], "wk_sb")
                wv_sb = load_whh(wv[l], "wv_sb")
                bq_sb = load_bias(bq[l], HT, "bq")
                bk_sb = load_bias(bk[l], HT, "bk")
                bo_sb = load_bias(bo2[l], HT, "bo")
                b1_sb = load_bias(b1[l], FFT, "b1")
                b2_sb = load_bias(b2[l], HT, "b2")

                # ======== attention ========
                with tc.tile_pool(name=f"attn{l}", bufs=1) as ap_, \
                     tc.tile_pool(name=f"attnps{l}", bufs=2, space="PSUM") as aps:

                    o_can = ap_.tile([P, HT, TOK], BF16, tag="ob", name="o_can")

                    for b in range(BPC):
                        bsl = bass.ds(b * S, S)
                        # --- q, k projections (transposed layout) ---
                        qT = ap_.tile([P, HT, S], BF16, tag="qT", bufs=2,
                                      name="qT")
                        kT = ap_.tile([P, HT, S], BF16, tag="kT", bufs=2,
                                      name="kT")
                        for dst, w_sb, bias_sb in ((qT, wq_sb, bq_sb),
                                                   (kT, wk_sb, bk_sb)):
                            for dd in range(HT):
                                ps = aps.tile([P, S], F32, tag="pp", name="pp_qk")
                                for k in range(HT):
                                    nc.tensor.matmul(
                                        ps[:], w_sb[:, k, bass.ts(dd, P)],
                                        x[:, k, bsl],
                                        start=(k == 0), stop=(k == HT - 1))
                                nc.scalar.activation(
                                    dst[:, dd, :], ps[:],
                                    mybir.ActivationFunctionType.Identity,
                                    bias=bias_sb[:, dd:dd + 1])
                        # --- v projection (token-major) + ones column ---
                        v_sb = ap_.tile([P, SB, NH, HD + 1], BF16, tag="v",
                                        bufs=2, name="v_sb")
                        nc.vector.memset(v_sb[:, :, :, HD:HD + 1], 1.0)
                        for tb in range(SB):
                            for nc2 in range(2):
                                ps = aps.tile([P, S], F32, tag="pp", name="pp_v")
                                for k in range(HT):
                                    nc.tensor.matmul(
                                        ps[:],
                                        x[:, k, bass.ds(b * S + tb * P, P)],
                                        wv_sb[:, k, bass.ds(nc2 * S, S)],
                                        start=(k == 0), stop=(k == HT - 1))
                                nc.vector.tensor_copy(
                                    out=v_sb[:, tb, bass.ds(nc2 * 6, 6), 0:HD],
                                    in_=ps[:].rearrange("p (h d) -> p h d", d=HD))

                        # --- scores + exp (no max; scores are O(1)) ---
                        wex = [None] * SB
                        for jb in range(SB):
                            ilo = jb * P          # only i >= jb*P matter
                            wex[jb] = ap_.tile([P, NH, S], BF16, tag="wexp",
                                               bufs=SB, name="wexp")
                            for h in range(NH):
                                po = (h % 2) * HD
                                kt = h // 2
                                ps = aps.tile([P, S], F32, tag="ps", name="ps_s")
                                nc.tensor.matmul(
                                    ps[:, ilo:],
                                    kT[po:po + HD, kt, bass.ds(ilo, P)],
                                    qT[po:po + HD, kt, ilo:],
                                    start=True, stop=True)
                                nc.scalar.activation(
                                    wex[jb][:, h, ilo:], ps[:, ilo:],
                                    mybir.ActivationFunctionType.Exp,
                                    scale=ATT_SCALE)
                            # causal mask on the diagonal 128-block only
                            nc.vector.tensor_mul(
                                wex[jb][:, :, bass.ds(ilo, P)],
                                wex[jb][:, :, bass.ds(ilo, P)],
                                mask_sb[:, jb, bass.ds(ilo, P)][:, None, :]
                                    .to_broadcast((P, NH, P)))

                        # --- PV + denominator + normalize ---
                        for h in range(NH):
                            po = (h % 2) * HD
                            kt = h // 2
                            pso = aps.tile([HD + 1, S], F32, tag="po", name="ps_o")
                            for jb in range(SB):
                                ilo = jb * P
                                nc.tensor.matmul(
                                    pso[:, ilo:], v_sb[:, jb, h, :],
                                    wex[jb][:, h, ilo:],
                                    start=(jb == 0), stop=(jb == SB - 1))
                            rr = rows.tile([1, S], F32, tag="rr", bufs=2,
                                           name="rr")
                            nc.vector.reciprocal(rr[:], pso[HD:HD + 1, :])
                            rr_bf = rows.tile([1, S], BF16, tag="rr_bf", bufs=2,
                                              name="rr_bf")
                            nc.scalar.copy(rr_bf[:], rr[:])
                            prb = aps.tile([HD, S], F32, tag="prb", bufs=1,
                                           name="ps_rb")
                            nc.tensor.matmul(prb[:], ones_row[:, 0:HD], rr_bf[:],
                                             start=True, stop=True)
                            rb_sb = rows.tile([HD, S], F32, tag="rb", bufs=2,
                                              name="rb_sb")
                            nc.scalar.copy(rb_sb[:], prb[:])
                            nc.vector.tensor_mul(
                                o_can[po:po + HD, kt, bsl],
                                pso[0:HD, :], rb_sb[:])

                    # ---- output projection + residual ----
                    wo_sb = load_whh(wo[l], "wo_sb")
                    x_res = xpool.tile([P, HT, TOK], BF16, tag="x",
                                       name="x_res1")
                    for b in range(BPC):
                        bsl = bass.ds(b * S, S)
                        for dd in range(HT):
                            ps = aps.tile([P, S], F32, tag="pp", name="pp_wo")
                            for k in range(HT):
                                nc.tensor.matmul(
                                    ps[:], wo_sb[:, k, bass.ts(dd, P)],
                                    o_can[:, k, bsl],
                                    start=(k == 0), stop=(k == HT - 1))
                            nc.scalar.activation(
                                ps[:], ps[:],
                                mybir.ActivationFunctionType.Identity,
                                bias=bo_sb[:, dd:dd + 1])
                            nc.vector.tensor_add(x_res[:, dd, bsl], ps[:],
                                                 x[:, dd, bsl])

                x_ln = layernorm(l, 0, x_res, f"a{l}")

                # ======== MLP ========
                with tc.tile_pool(name=f"mlp{l}", bufs=1) as mp, \
                     tc.tile_pool(name=f"mlpps{l}", bufs=4, space="PSUM") as mps:
                    x_res2 = xpool.tile([P, HT, TOK], BF16, tag="x", name="x_res2")

                    for cn in range(NCH):
                        sl = bass.ts(cn, CW)
                        w1h = [None, None]
                        w2h = [None, None]
                        for half in range(2):
                            w1h[half] = wbigp.tile([P, HT, FF // 2], BF16,
                                                   tag="wbig", name="w1h")
                            nc.sync.dma_start(
                                w1h[half][:],
                                w1[l, :, bass.ds(half * (FF // 2), FF // 2)]
                                  .rearrange("(ko ki) m -> ki ko m", ki=P))
                        g_sb = mp.tile([P, FFT, CW], BF16, tag="g", bufs=2,
                                       name="g_sb")
                        for mb in range(FFT):
                            half, mloc = divmod(mb, FFT // 2)
                            ps = mps.tile([P, CW], F32, tag="pm", name="pm1")
                            for k in range(HT):
                                nc.tensor.matmul(
                                    ps[:], w1h[half][:, k, bass.ts(mloc, P)],
                                    x_ln[:, k, sl],
                                    start=(k == 0), stop=(k == HT - 1))
                            nc.scalar.activation(
                                g_sb[:, mb, :], ps[:],
                                mybir.ActivationFunctionType.Gelu,
                                bias=b1_sb[:, mb:mb + 1])
                        for half in range(2):
                            w2h[half] = wbigp.tile([P, FFT // 2, H], BF16,
                                                   tag="wbig", name="w2h")
                            nc.sync.dma_start(
                                w2h[half][:],
                                w2[l, bass.ds(half * (FF // 2), FF // 2), :]
                                  .rearrange("(ko ki) m -> ki ko m", ki=P))
                        for dd in range(HT):
                            ps = mps.tile([P, CW], F32, tag="pm", name="pm2")
                            for k in range(FFT):
                                half, kloc = divmod(k, FFT // 2)
                                nc.tensor.matmul(
                                    ps[:], w2h[half][:, kloc, bass.ts(dd, P)],
                                    g_sb[:, k, :],
                                    start=(k == 0), stop=(k == FFT - 1))
                            nc.scalar.activation(
                                ps[:], ps[:],
                                mybir.ActivationFunctionType.Identity,
                                bias=b2_sb[:, dd:dd + 1])
                            nc.vector.tensor_add(x_res2[:, dd, sl], ps[:],
                                                 x_ln[:, dd, sl])

                fd = hout_t if l == L - 1 else None
                x = layernorm(l, 1, x_res2, f"m{l}", final_dma=fd)

    nc.compile()
    return nc


def _get_nc():
    if "nc" not in _CACHE:
        _CACHE["nc"] = _build()
    return _CACHE["nc"]


def _prep_weights(params):
    """Host-side: stack blk+exp params, cast, fold biases."""
    p = params
    blk, exp = p["blk"], p["exp"]

    def cat(name):
        return np.concatenate([np.asarray(blk[name], np.float32),
                               np.asarray(exp[name], np.float32)], axis=0)

    Wq, Wk, Wv, Wo = cat("Wq"), cat("Wk"), cat("Wv"), cat("Wo")
    W1, W2 = cat("W1"), cat("W2")
    bq_, bk_, bv_, bo_ = cat("bq"), cat("bk"), cat("bv"), cat("bo")
    b1_, b2_ = cat("b1"), cat("b2")
    # fold V bias through softmax (rows sum to 1) and O-projection:
    # (o + bv) @ Wo + bo == o @ Wo + (bv @ Wo + bo)
    bo2_ = np.einsum("lh,lhd->ld", bv_, Wo) + bo_

    ln_s = np.stack([cat("ln1_s"), cat("ln2_s")], axis=1)  # [L, 2, H]
    ln_b = np.stack([cat("ln1_b"), cat("ln2_b")], axis=1)

    ii = np.arange(S)
    jj = np.arange(P)
    maskT = np.zeros((SB, P, S), np.float32)
    for jb in range(SB):
        maskT[jb] = ((jb * P + jj)[:, None] <= ii[None, :]).astype(np.float32)

    return {
        "wq": Wq.astype(np_bf16), "wk": Wk.astype(np_bf16),
        "wv": Wv.astype(np_bf16), "wo": Wo.astype(np_bf16),
        "w1": W1.astype(np_bf16), "w2": W2.astype(np_bf16),
        "bq": bq_.astype(np.float32), "bk": bk_.astype(np.float32),
        "bo2": bo2_.astype(np.float32),
        "b1": b1_.astype(np.float32), "b2": b2_.astype(np.float32),
        "ln_s": ln_s.astype(np_bf16), "ln_b": ln_b.astype(np_bf16),
        "maskT": maskT.astype(np_bf16),
    }


def _embed(states, actions, rewards_to_go, timesteps, params):
    """Host front-end: embeddings + interleave + ln0 -> h0 [B, S, H] f32."""
    p = params
    s = (np.asarray(states, np.float32)
         - np.asarray(p["state_mean"], np.float32)) \
        / np.asarray(p["state_std"], np.float32)
    r = np.asarray(rewards_to_go, np.float32) / 1000.0
    ts_idx = np.asarray(timesteps).astype(np.int64)
    te = np.asarray(p["Wt"], np.float32)[ts_idx]                      # [B,T,H]
    se = s @ np.asarray(p["Ws"], np.float32) + np.asarray(p["bs"], np.float32) + te
    ae = np.asarray(actions, np.float32) @ np.asarray(p["Wa"], np.float32) \
        + np.asarray(p["ba"], np.float32) + te
    re = r @ np.asarray(p["Wr"], np.float32) + np.asarray(p["br"], np.float32) + te
    h = np.stack([re, se, ae], axis=2).reshape(B, S, H)
    m = h.mean(-1, keepdims=True)
    v = ((h - m) ** 2).mean(-1, keepdims=True)
    h = (h - m) / np.sqrt(v + EPS) * np.asarray(p["ln0_s"], np.float32) \
        + np.asarray(p["ln0_b"], np.float32)
    return h


def _run_device(h0, wmaps, trace=False):
    nc = _get_nc()
    in_maps = []
    for c in range(NCORES):
        x0 = h0[c * BPC:(c + 1) * BPC].reshape(TOK, H)
        x0T = np.ascontiguousarray(x0.T).astype(np_bf16)
        m = dict(wmaps)
        m["x0T"] = x0T
        in_maps.append(m)
    res = bass_utils.run_bass_kernel_spmd(nc, in_maps,
                                          core_ids=list(range(NCORES)),
                                          trace=trace)
    hT = [res.results[c]["houtT"] for c in range(NCORES)]
    h = np.concatenate([a.T.reshape(BPC, S, H) for a in hT], axis=0)
    return h, res


def kernel(states, actions, rewards_to_go, timesteps, task_id, params,
           _trace=False):
    p = params
    h0 = _embed(states, actions, rewards_to_go, timesteps, p)
    wmaps = _prep_weights(p)
    h, res = _run_device(h0, wmaps, trace=_trace)
    if _trace:
        kernel.last_result = res

    hr = h.reshape(B, T, 3, H)
    Wpa = np.asarray(p["Wpa"], np.float32)
    Wps = np.asarray(p["Wps"], np.float32)
    Wpr = np.asarray(p["Wpr"], np.float32)
    a = np.tanh(hr[:, :, 1] @ Wpa + np.asarray(p["bpa"], np.float32))
    lo = np.asarray(p["act_low"], np.float32)
    hi = np.asarray(p["act_high"], np.float32)
    a = lo + (a + 1.0) * (hi - lo) / 2.0
    sp = hr[:, :, 2] @ Wps + np.asarray(p["bps"], np.float32)
    rp = hr[:, :, 2] @ Wpr + np.asarray(p["bpr"], np.float32)
    return (sp.astype(np.float32), a.astype(np.float32), rp.astype(np.float32))


# revision 27
# speedup vs baseline: 9.1091x; 9.1091x over previous
"""DecisionTransformer forward on 8 TRN2 NeuronCores.

Strategy: data-parallel over batch (32 -> 4 per core), weights replicated.
Host computes the tiny embedding front-end (token/state/action/reward embeds,
interleave, ln0) and the tiny prediction heads; the 8 transformer blocks
(>99% of FLOPs) run on-device in one Bass/Tile kernel per core.

Device layout: activations are kept feature-major ("transposed"): xT[H, tok]
with H=768 on 6 partition-tiles of 128 and tok = 4 batches x 384 tokens.
All matmuls consume/produce this layout directly. LayerNorm (reduction over
the partition dim) uses ones-matmul stats + rank-1 PE broadcasts.
Attention uses scores in [j, i] orientation with exp-without-max (scores are
O(1) here), multiplicative causal mask on the diagonal block only, and a
ones-column in V to get softmax denominators for free.

The math shortcut vs the reference: the expert blocks' prompt tokens are
appended AFTER the sequence and causally masked, and their outputs are
discarded -> they cannot affect the kept outputs. So all 8 blocks are plain
causal blocks over 384 tokens.
"""

import numpy as np
import ml_dtypes

import concourse.bass as bass
import concourse.tile as tile
from concourse import bacc, mybir
from concourse import bass_utils

# ---- model constants (hardcoded per spec) ----
B, T = 32, 128
H = 768
NH = 12
HD = 64          # head dim
FF = 4 * H       # 3072
L = 8            # 6 blk + 2 expert blocks
S = 3 * T        # 384 tokens per sequence
NCORES = 8
BPC = B // NCORES          # 4 batches per core
TOK = BPC * S              # 1536 tokens per core
P = 128
HT = H // P                # 6 partition tiles for H
FFT = FF // P              # 24 partition tiles for FF
SB = S // P                # 3 j-blocks per sequence
EPS = 1e-5
ATT_SCALE = 1.0 / np.sqrt(HD)
NCH = 3                    # token chunks
CW = TOK // NCH            # 512

BF16 = mybir.dt.bfloat16
F32 = mybir.dt.float32
np_bf16 = ml_dtypes.bfloat16

_CACHE = {}


def _build():
    """Build + compile the per-core Bass program (same NEFF on all 8 cores)."""
    nc = bacc.Bacc("TRN2", target_bir_lowering=False, debug=False,
                   enable_asserts=True, num_devices=NCORES)

    # ---- DRAM parameters ----
    x0T = nc.dram_tensor("x0T", [H, TOK], BF16, kind="ExternalInput").ap()
    wq = nc.dram_tensor("wq", [L, H, H], BF16, kind="ExternalInput").ap()
    wk = nc.dram_tensor("wk", [L, H, H], BF16, kind="ExternalInput").ap()
    wv = nc.dram_tensor("wv", [L, H, H], BF16, kind="ExternalInput").ap()
    wo = nc.dram_tensor("wo", [L, H, H], BF16, kind="ExternalInput").ap()
    w1 = nc.dram_tensor("w1", [L, H, FF], BF16, kind="ExternalInput").ap()
    w2 = nc.dram_tensor("w2", [L, FF, H], BF16, kind="ExternalInput").ap()
    bq = nc.dram_tensor("bq", [L, H], F32, kind="ExternalInput").ap()
    bk = nc.dram_tensor("bk", [L, H], F32, kind="ExternalInput").ap()
    bo2 = nc.dram_tensor("bo2", [L, H], F32, kind="ExternalInput").ap()  # bv@Wo+bo
    b1 = nc.dram_tensor("b1", [L, FF], F32, kind="ExternalInput").ap()
    b2 = nc.dram_tensor("b2", [L, H], F32, kind="ExternalInput").ap()
    ln_s = nc.dram_tensor("ln_s", [L, 2, H], F32, kind="ExternalInput").ap()
    ln_b = nc.dram_tensor("ln_b", [L, 2, H], F32, kind="ExternalInput").ap()
    # multiplicative causal mask, [j-block, pj, i] (diagonal block only is used)
    maskT = nc.dram_tensor("maskT", [SB, P, S], BF16, kind="ExternalInput").ap()
    hout = nc.dram_tensor("houtT", [H, TOK], F32, kind="ExternalOutput").ap()

    x0T_t = x0T.rearrange("(t p) n -> p t n", p=P)
    hout_t = hout.rearrange("(t p) n -> p t n", p=P)

    with tile.TileContext(nc) as tc:
        with tc.tile_pool(name="glob", bufs=1) as glob, \
             tc.tile_pool(name="xpool", bufs=2) as xpool, \
             tc.tile_pool(name="whh", bufs=3) as whhp, \
             tc.tile_pool(name="wbig", bufs=2) as wbigp, \
             tc.tile_pool(name="rows", bufs=1) as rows:

            # constants
            ones_col = glob.tile([P, 1], BF16, tag="onec")
            nc.vector.memset(ones_col[:], 1.0)
            ones_row = glob.tile([1, TOK], BF16, tag="oner")
            nc.vector.memset(ones_row[:], 1.0)
            mask_sb = glob.tile([P, SB, S], BF16, tag="mask")
            nc.sync.dma_start(mask_sb[:], maskT.rearrange("j p i -> p j i"))
            eps_c = glob.tile([1, 1], F32, tag="eps")
            nc.vector.memset(eps_c[:], EPS)

            # initial activations
            x = xpool.tile([P, HT, TOK], BF16, tag="x")
            nc.sync.dma_start(x[:], x0T_t)

            def load_whh(ap_l, nm):
                t = whhp.tile([P, HT, H], BF16, tag="whh", name=nm)
                nc.sync.dma_start(t[:], ap_l.rearrange("(ko ki) m -> ki ko m", ki=P))
                return t

            def load_bias(ap_l, n, nm):  # [n*P] f32 -> [P, n]
                t = rows.tile([P, n], F32, tag=f"b_{nm}", bufs=2, name=nm)
                nc.sync.dma_start(t[:], ap_l.rearrange("(t p) -> p t", p=P))
                return t

            def layernorm(l, which, x_in, ctx_name, final_dma=None):
                """x_in: [P, HT, TOK] bf16 -> new x tile (normalized, *s+b)."""
                s_col = load_bias(ln_s[l, which], HT, "lns")
                b_col = load_bias(ln_b[l, which], HT, "lnb")
                r_bf = rows.tile([1, TOK], BF16, tag="r_bf", bufs=1, name="r_bf")
                mr_bf = rows.tile([1, TOK], BF16, tag="mr_bf", bufs=1,
                                  name="mr_bf")

                with tc.tile_pool(name=f"ln{ctx_name}", bufs=1) as lnp, \
                     tc.tile_pool(name=f"lnps{ctx_name}", bufs=2,
                                  space="PSUM") as lnps:
                    sq = lnp.tile([P, HT, TOK], BF16, tag="sq")
                    if final_dma is None:
                        x_out = xpool.tile([P, HT, TOK], BF16, tag="x",
                                           name="x_ln")
                    else:
                        x_out = lnp.tile([P, HT, TOK], F32, tag="xf32",
                                         name="x_f32")

                    for cn in range(NCH):
                        sl = bass.ts(cn, CW)
                        nc.scalar.square(sq[:, :, sl], x_in[:, :, sl])
                        ps0 = lnps.tile([1, CW], F32, tag="pst0", name="ps0")
                        ps1 = lnps.tile([1, CW], F32, tag="pst1", name="ps1")
                        for k in range(HT):
                            nc.tensor.matmul(ps0[:], ones_col[:], x_in[:, k, sl],
                                             start=(k == 0), stop=(k == HT - 1))
                        for k in range(HT):
                            nc.tensor.matmul(ps1[:], ones_col[:], sq[:, k, sl],
                                             start=(k == 0), stop=(k == HT - 1))
                        mu_c = rows.tile([1, CW], F32, tag="mu_c", bufs=1,
                                         name="mu_c")
                        var_c = rows.tile([1, CW], F32, tag="var_c", bufs=1,
                                          name="var_c")
                        rr_c = rows.tile([1, CW], F32, tag="rr_c", bufs=1,
                                         name="rr_c")
                        nc.scalar.mul(mu_c[:], ps0[:], 1.0 / H)
                        nc.vector.tensor_mul(var_c[:], mu_c[:], mu_c[:])
                        nc.scalar.activation(var_c[:], var_c[:],
                                             mybir.ActivationFunctionType.Copy,
                                             scale=-1.0)
                        nc.vector.scalar_tensor_tensor(
                            out=var_c[:], in0=ps1[:], scalar=1.0 / H,
                            in1=var_c[:], op0=mybir.AluOpType.mult,
                            op1=mybir.AluOpType.add)      # E[x^2] - mu^2
                        nc.scalar.activation(var_c[:], var_c[:],
                                             mybir.ActivationFunctionType.Sqrt,
                                             bias=eps_c[:])    # std
                        nc.vector.reciprocal_approx_fast(rr_c[:], var_c[:])
                        nc.scalar.copy(r_bf[:, sl], rr_c[:])
                        nc.vector.tensor_mul(mr_bf[:, sl], mu_c[:], rr_c[:])

                        # broadcast r and mu*r across partitions via PE
                        rb_ps = lnps.tile([P, CW], F32, tag="rbp", name="rb_ps")
                        nc.tensor.matmul(rb_ps[:], ones_row[:, 0:P],
                                         r_bf[:, sl], start=True, stop=True)
                        mrb_ps = lnps.tile([P, CW], F32, tag="mrbp",
                                           name="mrb_ps")
                        nc.tensor.matmul(mrb_ps[:], ones_row[:, 0:P],
                                         mr_bf[:, sl], start=True, stop=True)
                        for pt in range(HT):
                            nc.vector.tensor_mul(x_out[:, pt, sl],
                                                 x_in[:, pt, sl], rb_ps[:])
                            nc.vector.tensor_sub(x_out[:, pt, sl],
                                                 x_out[:, pt, sl], mrb_ps[:])
                            nc.scalar.activation(
                                x_out[:, pt, sl], x_out[:, pt, sl],
                                mybir.ActivationFunctionType.Identity,
                                bias=b_col[:, pt:pt + 1],
                                scale=s_col[:, pt:pt + 1])
                    if final_dma is not None:
                        nc.sync.dma_start(final_dma, x_out[:])
                        return None
                return x_out

            for l in range(L):
                # ---- weights for this block ----
                wq_sb = load_whh(wq[l], "wq_sb")
                wk_sb = load_whh(wk[l], "wk_sb")
                wv_sb = load_whh(wv[l], "wv_sb")
                bq_sb = load_bias(bq[l], HT, "bq")
                bk_sb = load_bias(bk[l], HT, "bk")
                bo_sb = load_bias(bo2[l], HT, "bo")
                b1_sb = load_bias(b1[l], FFT, "b1")
                b2_sb = load_bias(b2[l], HT, "b2")

                # ======== attention ========
                with tc.tile_pool(name=f"attn{l}", bufs=1) as ap_, \
                     tc.tile_pool(name=f"attnps{l}", bufs=2, space="PSUM") as aps:

                    o_can = ap_.tile([P, HT, TOK], BF16, tag="ob", name="o_can")

                    for b in range(BPC):
                        bsl = bass.ds(b * S, S)
                        # --- q, k projections (transposed layout) ---
                        qT = ap_.tile([P, HT, S], BF16, tag="qT", bufs=2,
                                      name="qT")
                        kT = ap_.tile([P, HT, S], BF16, tag="kT", bufs=2,
                                      name="kT")
                        for dst, w_sb, bias_sb in ((qT, wq_sb, bq_sb),
                                                   (kT, wk_sb, bk_sb)):
                            for dd in range(HT):
                                ps = aps.tile([P, S], F32, tag="pp", name="pp_qk")
                                for k in range(HT):
                                    nc.tensor.matmul(
                                        ps[:], w_sb[:, k, bass.ts(dd, P)],
                                        x[:, k, bsl],
                                        start=(k == 0), stop=(k == HT - 1))
                                nc.scalar.activation(
                                    dst[:, dd, :], ps[:],
                                    mybir.ActivationFunctionType.Identity,
                                    bias=bias_sb[:, dd:dd + 1])
                        # --- v projection (token-major) + ones column ---
                        v_sb = ap_.tile([P, SB, NH, HD + 1], BF16, tag="v",
                                        bufs=2, name="v_sb")
                        nc.vector.memset(v_sb[:, :, :, HD:HD + 1], 1.0)
                        for tb in range(SB):
                            for nc2 in range(2):
                                ps = aps.tile([P, S], F32, tag="pp", name="pp_v")
                                for k in range(HT):
                                    nc.tensor.matmul(
                                        ps[:],
                                        x[:, k, bass.ds(b * S + tb * P, P)],
                                        wv_sb[:, k, bass.ds(nc2 * S, S)],
                                        start=(k == 0), stop=(k == HT - 1))
                                nc.vector.tensor_copy(
                                    out=v_sb[:, tb, bass.ds(nc2 * 6, 6), 0:HD],
                                    in_=ps[:].rearrange("p (h d) -> p h d", d=HD))

                        # --- scores + exp (no max; scores are O(1)) ---
                        wex = [None] * SB
                        for jb in range(SB):
                            ilo = jb * P          # only i >= jb*P matter
                            wex[jb] = ap_.tile([P, NH, S], BF16, tag="wexp",
                                               bufs=SB, name="wexp")
                            for h in range(NH):
                                po = (h % 2) * HD
                                kt = h // 2
                                ps = aps.tile([P, S], F32, tag="ps", name="ps_s")
                                nc.tensor.matmul(
                                    ps[:, ilo:],
                                    kT[po:po + HD, kt, bass.ds(ilo, P)],
                                    qT[po:po + HD, kt, ilo:],
                                    start=True, stop=True)
                                nc.scalar.activation(
                                    wex[jb][:, h, ilo:], ps[:, ilo:],
                                    mybir.ActivationFunctionType.Exp,
                                    scale=ATT_SCALE)
                            # causal mask on the diagonal 128-block only
                            nc.vector.tensor_mul(
                                wex[jb][:, :, bass.ds(ilo, P)],
                                wex[jb][:, :, bass.ds(ilo, P)],
                                mask_sb[:, jb, bass.ds(ilo, P)][:, None, :]
                                    .to_broadcast((P, NH, P)))

                        # --- PV + denominator + normalize ---
                        for h in range(NH):
                            po = (h % 2) * HD
                            kt = h // 2
                            pso = aps.tile([HD + 1, S], F32, tag="po", name="ps_o")
                            for jb in range(SB):
                                ilo = jb * P
                                nc.tensor.matmul(
                                    pso[:, ilo:], v_sb[:, jb, h, :],
                                    wex[jb][:, h, ilo:],
                                    start=(jb == 0), stop=(jb == SB - 1))
                            rr = rows.tile([1, S], F32, tag="rr", bufs=2,
                                           name="rr")
                            nc.vector.reciprocal_approx_fast(rr[:],
                                                             pso[HD:HD + 1, :])
                            rr_bf = rows.tile([1, S], BF16, tag="rr_bf", bufs=2,
                                              name="rr_bf")
                            nc.scalar.copy(rr_bf[:], rr[:])
                            prb = aps.tile([HD, S], F32, tag="prb", bufs=2,
                                           name="ps_rb")
                            nc.tensor.matmul(prb[:], ones_row[:, 0:HD], rr_bf[:],
                                             start=True, stop=True)
                            rb_sb = rows.tile([HD, S], F32, tag="rb", bufs=2,
                                              name="rb_sb")
                            nc.scalar.copy(rb_sb[:], prb[:])
                            nc.vector.tensor_mul(
                                o_can[po:po + HD, kt, bsl],
                                pso[0:HD, :], rb_sb[:])

                    # ---- output projection + residual ----
                    wo_sb = load_whh(wo[l], "wo_sb")
                    x_res = xpool.tile([P, HT, TOK], BF16, tag="x",
                                       name="x_res1")
                    for b in range(BPC):
                        bsl = bass.ds(b * S, S)
                        for dd in range(HT):
                            ps = aps.tile([P, S], F32, tag="pp", name="pp_wo")
                            for k in range(HT):
                                nc.tensor.matmul(
                                    ps[:], wo_sb[:, k, bass.ts(dd, P)],
                                    o_can[:, k, bsl],
                                    start=(k == 0), stop=(k == HT - 1))
                            nc.scalar.activation(
                                ps[:], ps[:],
                                mybir.ActivationFunctionType.Identity,
                                bias=bo_sb[:, dd:dd + 1])
                            nc.vector.tensor_add(x_res[:, dd, bsl], ps[:],
                                                 x[:, dd, bsl])

                x_ln = layernorm(l, 0, x_res, f"a{l}")

                # ======== MLP ========
                with tc.tile_pool(name=f"mlp{l}", bufs=1) as mp, \
                     tc.tile_pool(name=f"mlpps{l}", bufs=4, space="PSUM") as mps:
                    x_res2 = xpool.tile([P, HT, TOK], BF16, tag="x", name="x_res2")

                    for cn in range(NCH):
                        sl = bass.ts(cn, CW)
                        w1h = [None, None]
                        w2h = [None, None]
                        for half in range(2):
                            w1h[half] = wbigp.tile([P, HT, FF // 2], BF16,
                                                   tag="wbig", name="w1h")
                            nc.sync.dma_start(
                                w1h[half][:],
                                w1[l, :, bass.ds(half * (FF // 2), FF // 2)]
                                  .rearrange("(ko ki) m -> ki ko m", ki=P))
                        g_sb = mp.tile([P, FFT, CW], BF16, tag="g", bufs=2,
                                       name="g_sb")
                        for mb in range(FFT):
                            half, mloc = divmod(mb, FFT // 2)
                            ps = mps.tile([P, CW], F32, tag="pm", name="pm1")
                            for k in range(HT):
                                nc.tensor.matmul(
                                    ps[:], w1h[half][:, k, bass.ts(mloc, P)],
                                    x_ln[:, k, sl],
                                    start=(k == 0), stop=(k == HT - 1))
                            nc.scalar.activation(
                                g_sb[:, mb, :], ps[:],
                                mybir.ActivationFunctionType.Gelu,
                                bias=b1_sb[:, mb:mb + 1])
                        for half in range(2):
                            w2h[half] = wbigp.tile([P, FFT // 2, H], BF16,
                                                   tag="wbig", name="w2h")
                            nc.sync.dma_start(
                                w2h[half][:],
                                w2[l, bass.ds(half * (FF // 2), FF // 2), :]
                                  .rearrange("(ko ki) m -> ki ko m", ki=P))
                        for dd in range(HT):
                            ps = mps.tile([P, CW], F32, tag="pm", name="pm2")
                            for k in range(FFT):
                                half, kloc = divmod(k, FFT // 2)
                                nc.tensor.matmul(
                                    ps[:], w2h[half][:, kloc, bass.ts(dd, P)],
                                    g_sb[:, k, :],
                                    start=(k == 0), stop=(k == FFT - 1))
                            nc.scalar.activation(
                                ps[:], ps[:],
                                mybir.ActivationFunctionType.Identity,
                                bias=b2_sb[:, dd:dd + 1])
                            nc.vector.tensor_add(x_res2[:, dd, sl], ps[:],
                                                 x_ln[:, dd, sl])

                fd = hout_t if l == L - 1 else None
                x = layernorm(l, 1, x_res2, f"m{l}", final_dma=fd)

    nc.compile()
    return nc


def _get_nc():
    if "nc" not in _CACHE:
        _CACHE["nc"] = _build()
    return _CACHE["nc"]


def _prep_weights(params):
    """Host-side: stack blk+exp params, cast, fold biases."""
    p = params
    blk, exp = p["blk"], p["exp"]

    def cat(name):
        return np.concatenate([np.asarray(blk[name], np.float32),
                               np.asarray(exp[name], np.float32)], axis=0)

    Wq, Wk, Wv, Wo = cat("Wq"), cat("Wk"), cat("Wv"), cat("Wo")
    W1, W2 = cat("W1"), cat("W2")
    bq_, bk_, bv_, bo_ = cat("bq"), cat("bk"), cat("bv"), cat("bo")
    b1_, b2_ = cat("b1"), cat("b2")
    # fold V bias through softmax (rows sum to 1) and O-projection:
    # (o + bv) @ Wo + bo == o @ Wo + (bv @ Wo + bo)
    bo2_ = np.einsum("lh,lhd->ld", bv_, Wo) + bo_

    ln_s = np.stack([cat("ln1_s"), cat("ln2_s")], axis=1)  # [L, 2, H]
    ln_b = np.stack([cat("ln1_b"), cat("ln2_b")], axis=1)

    ii = np.arange(S)
    jj = np.arange(P)
    maskT = np.zeros((SB, P, S), np.float32)
    for jb in range(SB):
        maskT[jb] = ((jb * P + jj)[:, None] <= ii[None, :]).astype(np.float32)

    return {
        "wq": Wq.astype(np_bf16), "wk": Wk.astype(np_bf16),
        "wv": Wv.astype(np_bf16), "wo": Wo.astype(np_bf16),
        "w1": W1.astype(np_bf16), "w2": W2.astype(np_bf16),
        "bq": bq_.astype(np.float32), "bk": bk_.astype(np.float32),
        "bo2": bo2_.astype(np.float32),
        "b1": b1_.astype(np.float32), "b2": b2_.astype(np.float32),
        "ln_s": ln_s.astype(np.float32), "ln_b": ln_b.astype(np.float32),
        "maskT": maskT.astype(np_bf16),
    }


def _embed(states, actions, rewards_to_go, timesteps, params):
    """Host front-end: embeddings + interleave + ln0 -> h0 [B, S, H] f32."""
    p = params
    s = (np.asarray(states, np.float32)
         - np.asarray(p["state_mean"], np.float32)) \
        / np.asarray(p["state_std"], np.float32)
    r = np.asarray(rewards_to_go, np.float32) / 1000.0
    ts_idx = np.asarray(timesteps).astype(np.int64)
    te = np.asarray(p["Wt"], np.float32)[ts_idx]                      # [B,T,H]
    se = s @ np.asarray(p["Ws"], np.float32) + np.asarray(p["bs"], np.float32) + te
    ae = np.asarray(actions, np.float32) @ np.asarray(p["Wa"], np.float32) \
        + np.asarray(p["ba"], np.float32) + te
    re = r @ np.asarray(p["Wr"], np.float32) + np.asarray(p["br"], np.float32) + te
    h = np.stack([re, se, ae], axis=2).reshape(B, S, H)
    m = h.mean(-1, keepdims=True)
    v = ((h - m) ** 2).mean(-1, keepdims=True)
    h = (h - m) / np.sqrt(v + EPS) * np.asarray(p["ln0_s"], np.float32) \
        + np.asarray(p["ln0_b"], np.float32)
    return h


def _run_device(h0, wmaps, trace=False):
    nc = _get_nc()
    in_maps = []
    for c in range(NCORES):
        x0 = h0[c * BPC:(c + 1) * BPC].reshape(TOK, H)
        x0T = np.ascontiguousarray(x0.T).astype(np_bf16)
        m = dict(wmaps)
        m["x0T"] = x0T
        in_maps.append(m)
    res = bass_utils.run_bass_kernel_spmd(nc, in_maps,
                                          core_ids=list(range(NCORES)),
                                          trace=trace)
    hT = [res.results[c]["houtT"] for c in range(NCORES)]
    h = np.concatenate([a.T.reshape(BPC, S, H) for a in hT], axis=0)
    return h, res


def kernel(states, actions, rewards_to_go, timesteps, task_id, params,
           _trace=False):
    p = params
    h0 = _embed(states, actions, rewards_to_go, timesteps, p)
    wmaps = _prep_weights(p)
    h, res = _run_device(h0, wmaps, trace=_trace)
    if _trace:
        kernel.last_result = res

    hr = h.reshape(B, T, 3, H)
    Wpa = np.asarray(p["Wpa"], np.float32)
    Wps = np.asarray(p["Wps"], np.float32)
    Wpr = np.asarray(p["Wpr"], np.float32)
    a = np.tanh(hr[:, :, 1] @ Wpa + np.asarray(p["bpa"], np.float32))
    lo = np.asarray(p["act_low"], np.float32)
    hi = np.asarray(p["act_high"], np.float32)
    a = lo + (a + 1.0) * (hi - lo) / 2.0
    sp = hr[:, :, 2] @ Wps + np.asarray(p["bps"], np.float32)
    rp = hr[:, :, 2] @ Wpr + np.asarray(p["bpr"], np.float32)
    return (sp.astype(np.float32), a.astype(np.float32), rp.astype(np.float32))
